# revision 1
# baseline (speedup 1.0000x reference)
"""Longformer sliding-window self-attention (MBart variant) on 8 TRN2 cores.

Strategy: sequence-parallel sharding. Each of the 8 cores gets one
(batch, quarter-sequence) shard: core c -> batch c//4, queries
[1024*(c%4), 1024*(c%4+1)). Each core receives a halo'd slice of the
hidden states (1536 rows, transposed, zero-padded at sequence edges),
computes Q/K/V projections, banded attention over 4 chunks of 256
queries x 768-key windows, and the output projection, returning its
[768, 1024] transposed output slice. Host re-assembles the full
[2, 4096, 768] output.

All matmuls run as float32r (full PE rate, ~1e-4 component error).

Math notes (exact rewrites of the reference):
  - query scale 1/sqrt(64) folded into Wq/bq on host.
  - Wk bias drops out of softmax entirely (constant per query row).
  - Wv bias commutes through softmax (weights sum to 1) and the output
    projection: folded into bo_eff = bo + Wo @ bv on host.
  - band + sequence-edge + attention_mask key bias folded into additive
    per-core mask tiles built on host (NEG = -1e9 outside the band).
  - softmax computed unnormalized; the denominator is produced by an
    extra all-ones column appended to each head's V block, and applied
    as a reciprocal multiply on the context rows.
"""

import numpy as np

# problem shapes (fixed by the task)
B, S, D, H = 2, 4096, 768, 12
DH = D // H            # 64
W = 256                # one-sided window == chunk size b
NEG = -1e9
NCORES = 8
G = 4                  # sequence groups per batch (8 cores / 2 batches)
SLOC = S // G          # 1024 queries per core
SH = SLOC + 2 * W      # 1536 halo'd rows per core
NB = SLOC // W         # 4 chunks per core
NT = 3 * W // 128      # 6 key tiles of 128 per chunk window
P = 128
DJ = D // P            # 6 tiles of 128 over the model dim

_PROGRAM_CACHE: dict = {}


def _build_program(general_mask: bool):
    import concourse.bacc as bacc
    import concourse.mybir as mybir
    import concourse.tile as tile
    from contextlib import ExitStack

    F32 = mybir.dt.float32
    F32R = mybir.dt.float32r
    AF = mybir.ActivationFunctionType
    NS = 3 if general_mask else 2        # mask slots per chunk
    MCOLS = NB * NS * 512                # mask sbuf columns

    nc = bacc.Bacc("TRN2", target_bir_lowering=False, debug=False)

    hsT = nc.dram_tensor("hsT", [D, SH], F32R, kind="ExternalInput")
    wqT = nc.dram_tensor("wqT", [D, D], F32R, kind="ExternalInput")
    wkT = nc.dram_tensor("wkT", [D, D], F32R, kind="ExternalInput")
    wvT = nc.dram_tensor("wvT", [D, D], F32R, kind="ExternalInput")
    woT = nc.dram_tensor("woT", [D, D], F32R, kind="ExternalInput")
    bq = nc.dram_tensor("bq", [D], F32, kind="ExternalInput")
    boe = nc.dram_tensor("boe", [D], F32, kind="ExternalInput")
    masks = nc.dram_tensor("masks", [NB, NS, P, 512], F32, kind="ExternalInput")
    outT = nc.dram_tensor("outT", [D, SLOC], F32, kind="ExternalOutput")

    with tile.TileContext(nc) as tc, ExitStack() as stack:
        const = stack.enter_context(tc.tile_pool(name="const", bufs=1))
        qt_p = stack.enter_context(tc.tile_pool(name="qt", bufs=1))
        kt_p = stack.enter_context(tc.tile_pool(name="kt", bufs=1))
        ct_p = stack.enter_context(tc.tile_pool(name="ct", bufs=1))

        bq_sb = const.tile([P, DJ], F32, tag="bq")
        nc.sync.dma_start(out=bq_sb[:], in_=bq.rearrange("(t p) -> p t", p=P))
        boe_sb = const.tile([P, DJ], F32, tag="boe")
        nc.sync.dma_start(out=boe_sb[:], in_=boe.rearrange("(t p) -> p t", p=P))
        mask_sb = const.tile([P, MCOLS], F32, tag="masks")
        for n in range(NB):
            for sl in range(NS):
                off = (n * NS + sl) * 512
                nc.sync.dma_start(
                    out=mask_sb[:, off : off + 512], in_=masks[n, sl]
                )

        QT = [qt_p.tile([P, SLOC], F32R, tag=f"qt{j}", name=f"qt{j}") for j in range(DJ)]
        KT = [kt_p.tile([P, SH], F32R, tag=f"kt{j}", name=f"kt{j}") for j in range(DJ)]
        CT = [ct_p.tile([P, SLOC], F32R, tag=f"ct{j}", name=f"ct{j}") for j in range(DJ)]

        # ---------------- phase 1: projections ------------------------
        # pool stack is LIFO: va (persistent) must open before hs.
        va_p = stack.enter_context(tc.tile_pool(name="va", bufs=1))
        VA = [va_p.tile([P, H * (DH + 1)], F32R, tag=f"va{s}", name=f"va{s}") for s in range(SH // P)]

        ones_f = const.tile([P, H], F32, tag="ones_f")
        nc.vector.memset(ones_f[:], 1.0)
        for st in range(SH // P):
            view = VA[st].rearrange("p (h e) -> p h e", e=DH + 1)
            nc.vector.tensor_copy(view[:, :, DH : DH + 1], ones_f[:])

        hs_stack = ExitStack()
        hs_p = hs_stack.enter_context(tc.tile_pool(name="hs", bufs=1))
        HS = [hs_p.tile([P, SH], F32R, tag=f"hs{i}", name=f"hs{i}") for i in range(DJ)]
        for i in range(DJ):
            nc.sync.dma_start(out=HS[i][:], in_=hsT[P * i : P * (i + 1), :])

        # V = hs @ Wv.T  -> VA[st] tiles [128 seq, 768 head-dims]
        with (
            tc.tile_pool(name="wv", bufs=1) as wv_p,
            tc.tile_pool(name="ps2", bufs=4, space="PSUM") as ps2,
        ):
            for half in range(2):
                WV = [wv_p.tile([P, 384], F32R, tag=f"wv{i}", name=f"wv{i}") for i in range(DJ)]
                for i in range(DJ):
                    nc.sync.dma_start(
                        out=WV[i][:],
                        in_=wvT[P * i : P * (i + 1), 384 * half : 384 * (half + 1)],
                    )
                for st in range(SH // P):
                    ps = ps2.tile([P, 384], F32, tag="ps2")
                    for i in range(DJ):
                        nc.tensor.matmul(
                            ps[:],
                            HS[i][:, P * st : P * (st + 1)],
                            WV[i][:],
                            start=(i == 0),
                            stop=(i == DJ - 1),
                        )
                    view = VA[st].rearrange("p (h e) -> p h e", e=DH + 1)
                    nc.vector.tensor_copy(
                        view[:, 6 * half : 6 * (half + 1), 0:DH],
                        ps[:].rearrange("p (h e) -> p h e", e=DH),
                    )

        # QT[j] = (Wq/8) @ hs_loc.T + bq/8
        with (
            tc.tile_pool(name="wq", bufs=1) as wq_p,
            tc.tile_pool(name="ps1", bufs=4, space="PSUM") as ps1,
        ):
            for half in range(2):
                WQ = [wq_p.tile([P, 384], F32R, tag=f"wq{i}", name=f"wq{i}") for i in range(DJ)]
                for i in range(DJ):
                    nc.sync.dma_start(
                        out=WQ[i][:],
                        in_=wqT[P * i : P * (i + 1), 384 * half : 384 * (half + 1)],
                    )
                for j in range(3 * half, 3 * half + 3):
                    jc = P * j - 384 * half
                    for sp in range(SLOC // 512):
                        ps = ps1.tile([P, 512], F32, tag="ps1")
                        for i in range(DJ):
                            nc.tensor.matmul(
                                ps[:],
                                WQ[i][:, jc : jc + P],
                                HS[i][:, W + 512 * sp : W + 512 * (sp + 1)],
                                start=(i == 0),
                                stop=(i == DJ - 1),
                            )
                        nc.scalar.activation(
                            QT[j][:, 512 * sp : 512 * (sp + 1)],
                            ps[:],
                            AF.Identity,
                            bias=bq_sb[:, j : j + 1],
                        )

        # KT[j] = Wk @ hs_halo.T  (bias bk cancels in softmax)
        with (
            tc.tile_pool(name="wk", bufs=1) as wk_p,
            tc.tile_pool(name="ps1b", bufs=4, space="PSUM") as ps1b,
        ):
            for half in range(2):
                WK = [wk_p.tile([P, 384], F32R, tag=f"wk{i}", name=f"wk{i}") for i in range(DJ)]
                for i in range(DJ):
                    nc.sync.dma_start(
                        out=WK[i][:],
                        in_=wkT[P * i : P * (i + 1), 384 * half : 384 * (half + 1)],
                    )
                for j in range(3 * half, 3 * half + 3):
                    jc = P * j - 384 * half
                    for sp in range(SH // 512):
                        ps = ps1b.tile([P, 512], F32, tag="ps1b")
                        for i in range(DJ):
                            nc.tensor.matmul(
                                ps[:],
                                WK[i][:, jc : jc + P],
                                HS[i][:, 512 * sp : 512 * (sp + 1)],
                                start=(i == 0),
                                stop=(i == DJ - 1),
                            )
                        nc.scalar.activation(
                            KT[j][:, 512 * sp : 512 * (sp + 1)], ps[:], AF.Copy
                        )
        hs_stack.close()

        # ---------------- phase 2: banded attention -------------------
        with (
            tc.tile_pool(name="expp", bufs=3) as exp_p,
            tc.tile_pool(name="dn", bufs=4) as dn_p,
            tc.tile_pool(name="pss", bufs=2, space="PSUM") as pss,
            tc.tile_pool(name="psc", bufs=2, space="PSUM") as psc,
        ):
            for n in range(NB):
                for j in range(DJ):
                    sps = [pss.tile([P, NT * W], F32, tag="s", name=f"s{n}_{j}_{k}") for k in range(2)]
                    for t in range(NT):
                        for hh in range(2):
                            r0 = DH * hh
                            nc.tensor.matmul(
                                sps[hh][:, W * t : W * (t + 1)],
                                KT[j][r0 : r0 + DH, W * n + P * t : W * n + P * (t + 1)],
                                QT[j][r0 : r0 + DH, W * n : W * (n + 1)],
                                start=True,
                                stop=True,
                            )
                    for hh in range(2):
                        h = 2 * j + hh
                        moff = n * NS * 512
                        nc.vector.tensor_add(
                            sps[hh][:, 0:512],
                            sps[hh][:, 0:512],
                            mask_sb[:, moff : moff + 512],
                        )
                        if general_mask:
                            nc.vector.tensor_add(
                                sps[hh][:, 512:1024],
                                sps[hh][:, 512:1024],
                                mask_sb[:, moff + 512 : moff + 1024],
                            )
                        lastoff = moff + (NS - 1) * 512
                        nc.vector.tensor_add(
                            sps[hh][:, 1024:1536],
                            sps[hh][:, 1024:1536],
                            mask_sb[:, lastoff : lastoff + 512],
                        )
                        expt = exp_p.tile([P, NT * W], F32R, tag="e", name=f"e{n}_{j}_{hh}")
                        nc.scalar.activation(expt[:], sps[hh][:], AF.Exp)
                        cps = psc.tile([DH + 1, W], F32, tag="c", name=f"c{n}_{h}")
                        for t in range(NT):
                            nc.tensor.matmul(
                                cps[:],
                                VA[2 * n + t][:, (DH + 1) * h : (DH + 1) * (h + 1)],
                                expt[:, W * t : W * (t + 1)],
                                start=(t == 0),
                                stop=(t == NT - 1),
                            )
                        # denominator row -> partition 0 (DMA hop), reciprocal,
                        # broadcast over the head's 64 lanes, fused normalize
                        dcp = dn_p.tile([DH + 1, W], F32, tag="dcp", name=f"dcp{n}_{h}")
                        nc.vector.tensor_copy(dcp[DH : DH + 1, :], cps[DH : DH + 1, :])
                        dnrow = dn_p.tile([1, W], F32, tag="dnr", name=f"dnr{n}_{h}")
                        nc.sync.dma_start(out=dnrow[:], in_=dcp[DH : DH + 1, :])
                        rcrow = dn_p.tile([1, W], F32, tag="rcr", name=f"rcr{n}_{h}")
                        scr = dn_p.tile([1, W], F32, tag="scr", name=f"scr{n}_{h}")
                        nc.vector.reciprocal_approx_accurate(
                            out=rcrow[:], in_=dnrow[:], scratch=scr[:]
                        )
                        rb = dn_p.tile([DH, W], F32, tag="rb", name=f"rb{n}_{h}")
                        nc.gpsimd.partition_broadcast(rb[:], rcrow[:], channels=DH)
                        if hh == 0:
                            nc.vector.tensor_mul(
                                CT[j][0:DH, W * n : W * (n + 1)], cps[0:DH, :], rb[:]
                            )
                        else:
                            stg = dn_p.tile([DH, W], F32R, tag="stg", name=f"stg{n}_{h}")
                            nc.vector.tensor_mul(stg[:], cps[0:DH, :], rb[:])
                            nc.sync.dma_start(
                                out=CT[j][DH:P, W * n : W * (n + 1)], in_=stg[:]
                            )

        # ---------------- phase 3: output projection ------------------
        with (
            tc.tile_pool(name="wo", bufs=1) as wo_p,
            tc.tile_pool(name="ob", bufs=3) as ob_p,
            tc.tile_pool(name="ps3", bufs=4, space="PSUM") as ps3,
        ):
            WO = [wo_p.tile([P, D], F32R, tag=f"wo{i}", name=f"wo{i}") for i in range(DJ)]
            for i in range(DJ):
                nc.sync.dma_start(out=WO[i][:], in_=woT[P * i : P * (i + 1), :])
            for j in range(DJ):
                for sp in range(SLOC // 512):
                    ps = ps3.tile([P, 512], F32, tag="ps3")
                    for i in range(DJ):
                        nc.tensor.matmul(
                            ps[:],
                            WO[i][:, P * j : P * (j + 1)],
                            CT[i][:, 512 * sp : 512 * (sp + 1)],
                            start=(i == 0),
                            stop=(i == DJ - 1),
                        )
                    osb = ob_p.tile([P, 512], F32, tag="ob")
                    nc.scalar.activation(
                        osb[:], ps[:], AF.Identity, bias=boe_sb[:, j : j + 1]
                    )
                    nc.sync.dma_start(
                        out=outT[P * j : P * (j + 1), 512 * sp : 512 * (sp + 1)],
                        in_=osb[:],
                    )

    nc.compile()
    return nc


def _host_prep(hidden_states, attention_mask, Wq, bq, Wk, bk, Wv, bv, Wo, bo):
    """Build per-core input maps. Returns (in_maps, general_mask)."""
    hs = np.asarray(hidden_states, dtype=np.float32)
    am = np.asarray(attention_mask, dtype=np.float32)
    Wq = np.asarray(Wq, dtype=np.float32)
    Wk = np.asarray(Wk, dtype=np.float32)
    Wv = np.asarray(Wv, dtype=np.float32)
    Wo = np.asarray(Wo, dtype=np.float32)
    bq = np.asarray(bq, dtype=np.float32)
    bv = np.asarray(bv, dtype=np.float32)
    bo = np.asarray(bo, dtype=np.float32)

    general = bool(np.any(am != 0.0))
    NS = 3 if general else 2
    scale = 1.0 / np.sqrt(np.float32(DH))

    wqT = np.ascontiguousarray(Wq.T * scale)
    wkT = np.ascontiguousarray(Wk.T)
    wvT = np.ascontiguousarray(Wv.T)
    woT = np.ascontiguousarray(Wo.T)
    bq_s = (bq * scale).astype(np.float32)
    bo_eff = (bo + Wo @ bv).astype(np.float32)

    # band validity per (tile t, partition p, q): kpos_w = 128 t + p
    t_idx = np.arange(NT)[:, None, None]
    p_idx = np.arange(P)[None, :, None]
    q_idx = np.arange(W)[None, None, :]
    kpos_w = P * t_idx + p_idx                      # [6,128,1]
    band_ok = np.abs(kpos_w - W - q_idx) <= W       # [6,128,256]

    in_maps = []
    for c in range(NCORES):
        bi, g = divmod(c, G)
        lo = SLOC * g - W
        halo = np.zeros((SH, D), dtype=np.float32)
        s0, s1 = max(lo, 0), min(lo + SH, S)
        halo[s0 - lo : s1 - lo] = hs[bi, s0:s1]
        hsT_c = np.ascontiguousarray(halo.T)

        m = np.empty((NB, NS, P, 512), dtype=np.float32)
        slot_tiles = [(0, 1), (2, 3), (4, 5)] if general else [(0, 1), (4, 5)]
        for n in range(NB):
            gc = NB * g + n                          # global chunk index
            kglob = W * gc + kpos_w - W              # [6,128,1]
            inb = (kglob >= 0) & (kglob < S)
            if general:
                kb = np.where(
                    inb, -am[bi, np.clip(kglob, 0, S - 1)], 0.0
                )                                    # [6,128,1] key bias
            else:
                kb = np.zeros_like(kglob, dtype=np.float32)
            valid = band_ok & inb
            mt = np.where(valid, kb, NEG).astype(np.float32)  # [6,128,256]
            for sl, (ta, tb) in enumerate(slot_tiles):
                m[n, sl, :, 0:256] = mt[ta]
                m[n, sl, :, 256:512] = mt[tb]

        in_maps.append(
            {
                "hsT": hsT_c,
                "wqT": wqT,
                "wkT": wkT,
                "wvT": wvT,
                "woT": woT,
                "bq": bq_s,
                "boe": bo_eff,
                "masks": m,
            }
        )
    return in_maps, general


def _run(inputs: dict, trace: bool = False):
    """Run the sharded kernel. Returns (full_output, BassKernelResults)."""
    from concourse.bass_utils import run_bass_kernel_spmd

    in_maps, general = _host_prep(**inputs)
    key = ("nc", general)
    if key not in _PROGRAM_CACHE:
        _PROGRAM_CACHE[key] = _build_program(general)
    nc = _PROGRAM_CACHE[key]

    res = run_bass_kernel_spmd(
        nc, in_maps, list(range(NCORES)), trace=trace
    )
    out = np.empty((B, S, D), dtype=np.float32)
    for c in range(NCORES):
        bi, g = divmod(c, G)
        out[bi, SLOC * g : SLOC * (g + 1), :] = res.results[c]["outT"].T
    return out, res


def kernel(**inputs) -> np.ndarray:
    out, _ = _run(inputs, trace=False)
    return out

